# revision 24
# baseline (speedup 1.0000x reference)
"""ActionCoherenceLoss kernel for 8 Trainium2 NeuronCores.

reference:
    norm = ||x||_2 along D; h = x / max(norm, eps)
    diag_sim[b, l] = <h[b,l], h[b,l+1]>          (l = 0..L-2)
    out = 1 - mean(diag_sim)                      (f32 scalar)

Strategy:
  - Data-parallel over batch: core b handles x[b] ([L=4096, D=2048]).
  - Host: transpose to x^T [D, L], pad one zero row -> [D, L+1], cast to
    fp8 e4m3 (final scalar rel-err ~3e-6), pack one 513-row slab of all 16
    feature chunks per DMA (contiguous ~1 MiB, 8.4 KiB per partition;
    row-stride padded to 528 so fp8 DoubleRow APs are 16B-aligned).
  - Device: for each 128-row block i, compute the near-diagonal Gram block
        G_i = X_blk^T @ X_blk'  in PSUM  ([128, 129], fp32 accum) with 8
    fp8 DoubleRow matmuls (256-deep contraction each).
        diag(G_i)[p] = s_{128i+p} = ||x_l||^2,
        superdiag(G_i)[p] = c_{128i+p} = <x_l, x_{l+1}>.
    Two blocks share one PSUM bank ([128, 258]) so the masked
    multiply+reduce extraction on VectorE amortizes per-op overhead.
  - Host: combine s, c from all 8 cores in float64:
        diag_sim_l = c_l / (max(sqrt(s_l),eps) * max(sqrt(s_{l+1}),eps))
"""

import numpy as np
import ml_dtypes

B, L, D = 8, 4096, 2048
P = 128
W = P + 1                      # 129: Gram block width (incl. superdiag col)
NCHUNK = D // P                # 16 feature chunks
NBLK = L // P                  # 32 Gram blocks per core
EPS = 1e-12
IN_DT = "float8e4"             # dtype in DRAM + SBUF: bfloat16 | float8e4
DOUBLE_ROW = True              # fp8 DoubleRow matmuls (half the MM count)

# Row slabs (DMA/pipeline granularity).  Small first slabs let the PE
# start early; each slab covers its blocks' rows plus one lookahead row.
SLAB_BLKS = [1, 3] + [4] * 6 + [3, 1]   # blocks per slab (sum = 32)
NSLAB = len(SLAB_BLKS)
SLAB_FIRST = [sum(SLAB_BLKS[:j]) for j in range(NSLAB)]      # first block
SLAB_NCOL = [128 * n + 1 for n in SLAB_BLKS]                 # valid cols
# per-chunk stored stride, %16==0 so fp8 DoubleRow APs are 16B-aligned
SLAB_STRIDE = [((c + 15) // 16) * 16 for c in SLAB_NCOL]
SLAB_OFF = [0]
for j in range(NSLAB - 1):
    SLAB_OFF.append(SLAB_OFF[-1] + P * NCHUNK * SLAB_STRIDE[j])
XT_SIZE = SLAB_OFF[-1] + P * NCHUNK * SLAB_STRIDE[-1]

_cache = {}


def _install_ntff_hook():
    """Best-effort: make antenv.axon_hooks importable so that
    run_bass_kernel_spmd's trace path (e.g. via BASS_TRACE=1) degrades
    gracefully instead of raising ImportError.  Registers the real
    libaxon NTFF hook when available, else a no-op."""
    import sys
    import types

    try:
        import antenv.axon_hooks  # noqa: F401

        return
    except ImportError:
        pass
    try:
        import antenv
    except ImportError:
        return
    mod = types.ModuleType("antenv.axon_hooks")
    mod._hook = None
    mod.set_axon_ntff_profile_hook = lambda h: setattr(mod, "_hook", h)
    mod.get_axon_ntff_profile_hook = lambda: mod._hook
    sys.modules["antenv.axon_hooks"] = mod
    antenv.axon_hooks = mod
    try:
        if "/root/.axon_site" not in sys.path:
            sys.path.insert(0, "/root/.axon_site")
        from trn_agent_boot.trn_boot import _ntff_profile_via_ctypes

        mod._hook = _ntff_profile_via_ctypes("/opt/axon/libaxon_pjrt.so")
    except Exception:
        pass


def _build():
    import concourse.bass as bass
    import concourse.bacc as bacc
    import concourse.tile as tile
    from concourse import mybir

    nc = bacc.Bacc("TRN2", target_bir_lowering=False, debug=False)
    f32 = mybir.dt.float32
    in_dt = getattr(mybir.dt, IN_DT)

    xt_d = nc.dram_tensor("xt", [XT_SIZE], in_dt, kind="ExternalInput").ap()
    bf16 = mybir.dt.bfloat16
    mk_d = nc.dram_tensor("mk", [P, 2 * 2 * W], bf16, kind="ExternalInput").ap()
    sc_d = nc.dram_tensor("sc", [2, P, NBLK], f32, kind="ExternalOutput").ap()

    with tile.TileContext(nc) as tc:
        with (
            tc.tile_pool(name="xin", bufs=1) as xin,
            tc.tile_pool(name="cst", bufs=1) as cst,
            tc.tile_pool(name="scr", bufs=4) as scr,
            tc.tile_pool(name="outp", bufs=1) as outp,
            tc.tile_pool(name="psum", bufs=8, space=bass.MemorySpace.PSUM) as psum,
        ):
            mk = cst.tile([P, 2 * 2 * W], bf16, name="mk_sb")
            sc = outp.tile([P, 2 * NBLK], f32, name="sc_sb")

            # One DMA per slab, all on one HWDGE queue so transfers finish
            # in issue order (slab 0 first, then the mask, then the rest).
            xt = []
            for j in range(NSLAB):
                t = xin.tile(
                    [P, NCHUNK * SLAB_STRIDE[j]], in_dt,
                    tag=f"xt_{j}", name=f"xt_{j}",
                )
                n = P * NCHUNK * SLAB_STRIDE[j]
                src_ap = xt_d[SLAB_OFF[j] : SLAB_OFF[j] + n].rearrange(
                    "(p c) -> p c", p=P
                )
                nc.sync.dma_start(out=t, in_=src_ap)
                xt.append(t)
                if j == 0:
                    nc.sync.dma_start(out=mk, in_=mk_d)

            # Blocks are processed in PSUM pairs (one bank holds two
            # Gram blocks).
            groups = [(2 * t, 2 * t + 1) for t in range(NBLK // 2)]
            for gi, grp in enumerate(groups):
                gw = len(grp) * W
                pb = psum.tile([P, gw], f32, tag="gram", name=f"gram_{gi}")
                for u, i in enumerate(grp):
                    j = max(jj for jj in range(NSLAB) if SLAB_FIRST[jj] <= i)
                    m0 = (i - SLAB_FIRST[j]) * P
                    t = xt[j]
                    out_ap = pb[:, u * W : (u + 1) * W]
                    if DOUBLE_ROW:
                        t3 = t[:].rearrange("p (c w) -> p c w", w=SLAB_STRIDE[j])
                        for k in range(NCHUNK // 2):
                            lhsT = t3[:, 2 * k : 2 * k + 2, m0 : m0 + P]
                            rhs = t3[:, 2 * k : 2 * k + 2, m0 : m0 + W]
                            nc.tensor.matmul(
                                out_ap, lhsT, rhs,
                                start=(k == 0),
                                stop=(k == NCHUNK // 2 - 1),
                                perf_mode=mybir.MatmulPerfMode.DoubleRow,
                            )
                    else:
                        for k in range(NCHUNK):
                            c0 = k * SLAB_STRIDE[j] + m0
                            nc.tensor.matmul(
                                out_ap,
                                t[:, c0 : c0 + P],
                                t[:, c0 : c0 + W],
                                start=(k == 0),
                                stop=(k == NCHUNK - 1),
                            )
                nb = len(grp)
                for h in range(2):  # 0 -> diag (s), 1 -> superdiag (c)
                    tmp = scr.tile(
                        [P, 2 * W], f32, tag="scr", name=f"scr_{gi}_{h}"
                    )
                    col = 2 * grp[0] + 2 * h  # [pair, h, u] layout
                    nc.vector.tensor_mul(
                        tmp[:, :gw], pb, mk[:, 2 * h * W : 2 * h * W + gw]
                    )
                    nc.vector.reduce_sum(
                        sc[:, col : col + nb],
                        tmp[:, :gw].rearrange("p (b c) -> p b c", b=nb),
                        axis=mybir.AxisListType.X,
                    )

                if gi == NBLK // 4 - 1:
                    nc.sync.dma_start(out=sc_d[0], in_=sc[:, :NBLK])
            nc.sync.dma_start(out=sc_d[1], in_=sc[:, NBLK:])
    nc.compile()
    return nc


def _make_masks():
    mk = np.zeros((P, 2, 2, W), np.float32)
    r = np.arange(P)
    mk[r, 0, :, r] = 1.0      # diag mask, replicated for both blocks
    mk[r, 1, :, r + 1] = 1.0  # superdiag mask
    return mk.reshape(P, 2 * 2 * W).astype(ml_dtypes.bfloat16)


def _np_in_dt():
    return {"float8e4": ml_dtypes.float8_e4m3, "bfloat16": ml_dtypes.bfloat16}[IN_DT]


def _prep_inputs(x):
    """x: [B, L, D] float32 -> (list of per-core input maps, scales)."""
    np_dt = _np_in_dt()
    mk = _make_masks()
    in_maps = []
    scales = []
    for b in range(B):
        # Scale into fp8 range (the loss is scale-invariant; the exact
        # factor is divided back out of s and c on the host).
        amax = float(np.max(np.abs(x[b])))
        alpha = (200.0 / amax) if (np.isfinite(amax) and amax > 0) else 1.0
        scales.append(alpha)
        xs = x[b].T * np.float32(alpha)
        xt = np.zeros((D, L + 1), dtype=np_dt)
        xt[:, :L] = np.ascontiguousarray(xs).astype(np_dt)
        arr = np.zeros(XT_SIZE, dtype=np_dt)
        for j in range(NSLAB):
            st, ncol = SLAB_STRIDE[j], SLAB_NCOL[j]
            r0 = SLAB_FIRST[j] * P
            sl = xt[:, r0 : r0 + ncol]                       # [D, ncol]
            a4 = arr[SLAB_OFF[j] : SLAB_OFF[j] + P * NCHUNK * st].reshape(
                P, NCHUNK, st
            )
            a4[:, :, :ncol] = sl.reshape(NCHUNK, P, ncol).transpose(1, 0, 2)
        in_maps.append({"xt": arr, "mk": mk})
    return in_maps, scales


def _combine(results, scales):
    total = 0.0
    for b in range(B):
        sc = np.asarray(results[b]["sc"], dtype=np.float64)  # [2, P, NBLK]
        sc = sc / (scales[b] * scales[b])
        q = sc.transpose(1, 0, 2).reshape(P, 2 * NBLK).reshape(
            P, NBLK // 2, 2, 2
        )  # [p, pair, h, u]
        s = q[:, :, 0, :].transpose(1, 2, 0).reshape(-1)  # l = 128*i + p
        c = q[:, :, 1, :].transpose(1, 2, 0).reshape(-1)
        n = np.maximum(np.sqrt(s), EPS)
        diag = c[: L - 1] / (n[: L - 1] * n[1:L])
        total += diag.sum()
    coherence = total / (B * (L - 1))
    return np.array(1.0 - coherence, dtype=np.float32)


def _run(x, trace=False):
    from concourse import bass_utils

    _install_ntff_hook()
    if "nc" not in _cache:
        _cache["nc"] = _build()
    nc = _cache["nc"]
    in_maps, scales = _prep_inputs(np.asarray(x, dtype=np.float32))
    res = bass_utils.run_bass_kernel_spmd(
        nc, in_maps, core_ids=list(range(B)), trace=trace
    )
    return _combine(res.results, scales), res


def kernel(hidden_states):
    out, _ = _run(hidden_states, trace=False)
    return out


# revision 25
# speedup vs baseline: 1.0070x; 1.0070x over previous
"""ActionCoherenceLoss kernel for 8 Trainium2 NeuronCores.

reference:
    norm = ||x||_2 along D; h = x / max(norm, eps)
    diag_sim[b, l] = <h[b,l], h[b,l+1]>          (l = 0..L-2)
    out = 1 - mean(diag_sim)                      (f32 scalar)

Strategy:
  - Data-parallel over batch: core b handles x[b] ([L=4096, D=2048]).
  - Host: transpose to x^T [D, L], pad one zero row -> [D, L+1], cast to
    fp8 e4m3 (final scalar rel-err ~3e-6), pack one 513-row slab of all 16
    feature chunks per DMA (contiguous ~1 MiB, 8.4 KiB per partition;
    row-stride padded to 528 so fp8 DoubleRow APs are 16B-aligned).
  - Device: for each 128-row block i, compute the near-diagonal Gram block
        G_i = X_blk^T @ X_blk'  in PSUM  ([128, 129], fp32 accum) with 8
    fp8 DoubleRow matmuls (256-deep contraction each).
        diag(G_i)[p] = s_{128i+p} = ||x_l||^2,
        superdiag(G_i)[p] = c_{128i+p} = <x_l, x_{l+1}>.
    Two blocks share one PSUM bank ([128, 258]) so the masked
    multiply+reduce extraction on VectorE amortizes per-op overhead.
  - Host: combine s, c from all 8 cores in float64:
        diag_sim_l = c_l / (max(sqrt(s_l),eps) * max(sqrt(s_{l+1}),eps))
"""

import numpy as np
import ml_dtypes

B, L, D = 8, 4096, 2048
P = 128
W = P + 1                      # 129: Gram block width (incl. superdiag col)
NCHUNK = D // P                # 16 feature chunks
NBLK = L // P                  # 32 Gram blocks per core
EPS = 1e-12
IN_DT = "float8e4"             # dtype in DRAM + SBUF: bfloat16 | float8e4
DOUBLE_ROW = True              # fp8 DoubleRow matmuls (half the MM count)

# Row slabs (DMA/pipeline granularity).  Small first slabs let the PE
# start early; each slab covers its blocks' rows plus one lookahead row.
SLAB_BLKS = [1, 3] + [4] * 6 + [3, 1]   # blocks per slab (sum = 32)
NSLAB = len(SLAB_BLKS)
SLAB_FIRST = [sum(SLAB_BLKS[:j]) for j in range(NSLAB)]      # first block
SLAB_NCOL = [128 * n + 1 for n in SLAB_BLKS]                 # valid cols
# per-chunk stored stride, %16==0 so fp8 DoubleRow APs are 16B-aligned
SLAB_STRIDE = [((c + 15) // 16) * 16 for c in SLAB_NCOL]
SLAB_OFF = [0]
for j in range(NSLAB - 1):
    SLAB_OFF.append(SLAB_OFF[-1] + P * NCHUNK * SLAB_STRIDE[j])
XT_SIZE = SLAB_OFF[-1] + P * NCHUNK * SLAB_STRIDE[-1]

_cache = {}


def _install_ntff_hook():
    """Best-effort: make antenv.axon_hooks importable so that
    run_bass_kernel_spmd's trace path (e.g. via BASS_TRACE=1) degrades
    gracefully instead of raising ImportError.  Registers the real
    libaxon NTFF hook when available, else a no-op."""
    import sys
    import types

    try:
        import antenv.axon_hooks  # noqa: F401

        return
    except ImportError:
        pass
    try:
        import antenv
    except ImportError:
        return
    mod = types.ModuleType("antenv.axon_hooks")
    mod._hook = None
    mod.set_axon_ntff_profile_hook = lambda h: setattr(mod, "_hook", h)
    mod.get_axon_ntff_profile_hook = lambda: mod._hook
    sys.modules["antenv.axon_hooks"] = mod
    antenv.axon_hooks = mod
    try:
        if "/root/.axon_site" not in sys.path:
            sys.path.insert(0, "/root/.axon_site")
        from trn_agent_boot.trn_boot import _ntff_profile_via_ctypes

        mod._hook = _ntff_profile_via_ctypes("/opt/axon/libaxon_pjrt.so")
    except Exception:
        pass


def _build():
    import concourse.bass as bass
    import concourse.bacc as bacc
    import concourse.tile as tile
    from concourse import mybir

    nc = bacc.Bacc("TRN2", target_bir_lowering=False, debug=False)
    f32 = mybir.dt.float32
    in_dt = getattr(mybir.dt, IN_DT)

    xt_d = nc.dram_tensor("xt", [XT_SIZE], in_dt, kind="ExternalInput").ap()
    bf16 = mybir.dt.bfloat16
    mk_d = nc.dram_tensor("mk", [P, 2 * 2 * W], bf16, kind="ExternalInput").ap()
    sc_d = nc.dram_tensor("sc", [2, P, NBLK], f32, kind="ExternalOutput").ap()

    with tile.TileContext(nc) as tc:
        with (
            tc.tile_pool(name="xin", bufs=1) as xin,
            tc.tile_pool(name="cst", bufs=1) as cst,
            tc.tile_pool(name="scr", bufs=4) as scr,
            tc.tile_pool(name="outp", bufs=1) as outp,
            tc.tile_pool(name="psum", bufs=8, space=bass.MemorySpace.PSUM) as psum,
        ):
            mk = cst.tile([P, 2 * 2 * W], bf16, name="mk_sb")
            sc = outp.tile([P, 2 * NBLK], f32, name="sc_sb")

            # One DMA per slab, all on one HWDGE queue so transfers finish
            # in issue order (slab 0 first, then the mask, then the rest).
            xt = []
            for j in range(NSLAB):
                t = xin.tile(
                    [P, NCHUNK * SLAB_STRIDE[j]], in_dt,
                    tag=f"xt_{j}", name=f"xt_{j}",
                )
                n = P * NCHUNK * SLAB_STRIDE[j]
                src_ap = xt_d[SLAB_OFF[j] : SLAB_OFF[j] + n].rearrange(
                    "(p c) -> p c", p=P
                )
                nc.sync.dma_start(out=t, in_=src_ap)
                xt.append(t)
                if j == 0:
                    nc.sync.dma_start(out=mk, in_=mk_d)

            # Blocks are processed in PSUM pairs (one bank holds two
            # Gram blocks).
            groups = [(2 * t, 2 * t + 1) for t in range(NBLK // 2)]
            for gi, grp in enumerate(groups):
                gw = len(grp) * W
                pb = psum.tile([P, gw], f32, tag="gram", name=f"gram_{gi}")
                for u, i in enumerate(grp):
                    j = max(jj for jj in range(NSLAB) if SLAB_FIRST[jj] <= i)
                    m0 = (i - SLAB_FIRST[j]) * P
                    t = xt[j]
                    out_ap = pb[:, u * W : (u + 1) * W]
                    if DOUBLE_ROW:
                        t3 = t[:].rearrange("p (c w) -> p c w", w=SLAB_STRIDE[j])
                        for k in range(NCHUNK // 2):
                            lhsT = t3[:, 2 * k : 2 * k + 2, m0 : m0 + P]
                            rhs = t3[:, 2 * k : 2 * k + 2, m0 : m0 + W]
                            nc.tensor.matmul(
                                out_ap, lhsT, rhs,
                                start=(k == 0),
                                stop=(k == NCHUNK // 2 - 1),
                                perf_mode=mybir.MatmulPerfMode.DoubleRow,
                            )
                    else:
                        for k in range(NCHUNK):
                            c0 = k * SLAB_STRIDE[j] + m0
                            nc.tensor.matmul(
                                out_ap,
                                t[:, c0 : c0 + P],
                                t[:, c0 : c0 + W],
                                start=(k == 0),
                                stop=(k == NCHUNK - 1),
                            )
                # One double-wide masked multiply covers the diag AND
                # superdiag masks (the PSUM pair is read twice via a
                # step-0 broadcast dim), then one 4-group reduce emits
                # [s_b0, s_b1, c_b0, c_b1] for this pair.
                pb_ap = pb[:]
                pb2 = bass.AP(
                    tensor=pb_ap.tensor,
                    offset=pb_ap.offset,
                    ap=[pb_ap.ap[0], [0, 2], pb_ap.ap[1]],
                )  # [P, 2, 2*W], middle dim broadcast
                tmp = scr.tile([P, 2, 2 * W], f32, tag="scr", name=f"scr_{gi}")
                nc.vector.tensor_mul(
                    tmp, pb2, mk[:].rearrange("p (h w) -> p h w", h=2)
                )
                col = 2 * grp[0]  # [pair, h, u] layout
                nc.vector.reduce_sum(
                    sc[:, col : col + 4],
                    tmp[:].rearrange("p h (u w) -> p h u w", u=2),
                    axis=mybir.AxisListType.X,
                )

                if gi == NBLK // 4 - 1:
                    nc.sync.dma_start(out=sc_d[0], in_=sc[:, :NBLK])
            nc.sync.dma_start(out=sc_d[1], in_=sc[:, NBLK:])
    nc.compile()
    return nc


def _make_masks():
    mk = np.zeros((P, 2, 2, W), np.float32)
    r = np.arange(P)
    mk[r, 0, :, r] = 1.0      # diag mask, replicated for both blocks
    mk[r, 1, :, r + 1] = 1.0  # superdiag mask
    return mk.reshape(P, 2 * 2 * W).astype(ml_dtypes.bfloat16)


def _np_in_dt():
    return {"float8e4": ml_dtypes.float8_e4m3, "bfloat16": ml_dtypes.bfloat16}[IN_DT]


def _prep_inputs(x):
    """x: [B, L, D] float32 -> (list of per-core input maps, scales)."""
    np_dt = _np_in_dt()
    mk = _make_masks()
    in_maps = []
    scales = []
    for b in range(B):
        # Scale into fp8 range (the loss is scale-invariant; the exact
        # factor is divided back out of s and c on the host).
        amax = float(np.max(np.abs(x[b])))
        alpha = (200.0 / amax) if (np.isfinite(amax) and amax > 0) else 1.0
        scales.append(alpha)
        xs = x[b].T * np.float32(alpha)
        xt = np.zeros((D, L + 1), dtype=np_dt)
        xt[:, :L] = np.ascontiguousarray(xs).astype(np_dt)
        arr = np.zeros(XT_SIZE, dtype=np_dt)
        for j in range(NSLAB):
            st, ncol = SLAB_STRIDE[j], SLAB_NCOL[j]
            r0 = SLAB_FIRST[j] * P
            sl = xt[:, r0 : r0 + ncol]                       # [D, ncol]
            a4 = arr[SLAB_OFF[j] : SLAB_OFF[j] + P * NCHUNK * st].reshape(
                P, NCHUNK, st
            )
            a4[:, :, :ncol] = sl.reshape(NCHUNK, P, ncol).transpose(1, 0, 2)
        in_maps.append({"xt": arr, "mk": mk})
    return in_maps, scales


def _combine(results, scales):
    total = 0.0
    for b in range(B):
        sc = np.asarray(results[b]["sc"], dtype=np.float64)  # [2, P, NBLK]
        sc = sc / (scales[b] * scales[b])
        q = sc.transpose(1, 0, 2).reshape(P, 2 * NBLK).reshape(
            P, NBLK // 2, 2, 2
        )  # [p, pair, h, u]
        s = q[:, :, 0, :].transpose(1, 2, 0).reshape(-1)  # l = 128*i + p
        c = q[:, :, 1, :].transpose(1, 2, 0).reshape(-1)
        n = np.maximum(np.sqrt(s), EPS)
        diag = c[: L - 1] / (n[: L - 1] * n[1:L])
        total += diag.sum()
    coherence = total / (B * (L - 1))
    return np.array(1.0 - coherence, dtype=np.float32)


def _run(x, trace=False):
    from concourse import bass_utils

    _install_ntff_hook()
    if "nc" not in _cache:
        _cache["nc"] = _build()
    nc = _cache["nc"]
    in_maps, scales = _prep_inputs(np.asarray(x, dtype=np.float32))
    res = bass_utils.run_bass_kernel_spmd(
        nc, in_maps, core_ids=list(range(B)), trace=trace
    )
    return _combine(res.results, scales), res


def kernel(hidden_states):
    out, _ = _run(hidden_states, trace=False)
    return out


# revision 26
# speedup vs baseline: 1.0450x; 1.0377x over previous
"""ActionCoherenceLoss kernel for 8 Trainium2 NeuronCores.

reference:
    norm = ||x||_2 along D; h = x / max(norm, eps)
    diag_sim[b, l] = <h[b,l], h[b,l+1]>          (l = 0..L-2)
    out = 1 - mean(diag_sim)                      (f32 scalar)

Strategy:
  - Data-parallel over batch: core b handles x[b] ([L=4096, D=2048]).
  - Host: transpose to x^T [D, L], pad one zero row -> [D, L+1], cast to
    fp8 e4m3 (final scalar rel-err ~3e-6), pack one 513-row slab of all 16
    feature chunks per DMA (contiguous ~1 MiB, 8.4 KiB per partition;
    row-stride padded to 528 so fp8 DoubleRow APs are 16B-aligned).
  - Device: for each 128-row block i, compute the near-diagonal Gram block
        G_i = X_blk^T @ X_blk'  in PSUM  ([128, 129], fp32 accum) with 8
    fp8 DoubleRow matmuls (256-deep contraction each).
        diag(G_i)[p] = s_{128i+p} = ||x_l||^2,
        superdiag(G_i)[p] = c_{128i+p} = <x_l, x_{l+1}>.
    Two blocks share one PSUM bank ([128, 258]) so the masked
    multiply+reduce extraction on VectorE amortizes per-op overhead.
  - Host: combine s, c from all 8 cores in float64:
        diag_sim_l = c_l / (max(sqrt(s_l),eps) * max(sqrt(s_{l+1}),eps))
"""

import numpy as np
import ml_dtypes

B, L, D = 8, 4096, 2048
P = 128
W = P + 1                      # 129: Gram block width (incl. superdiag col)
NCHUNK = D // P                # 16 feature chunks
NBLK = L // P                  # 32 Gram blocks per core
EPS = 1e-12
IN_DT = "float8e4"             # dtype in DRAM + SBUF: bfloat16 | float8e4
DOUBLE_ROW = True              # fp8 DoubleRow matmuls (half the MM count)

# Row slabs (DMA/pipeline granularity).  Small first slabs let the PE
# start early; each slab covers its blocks' rows plus one lookahead row.
SLAB_BLKS = [1, 3] + [4] * 6 + [3, 1]   # blocks per slab (sum = 32)
NSLAB = len(SLAB_BLKS)
SLAB_FIRST = [sum(SLAB_BLKS[:j]) for j in range(NSLAB)]      # first block
SLAB_NCOL = [128 * n + 1 for n in SLAB_BLKS]                 # valid cols
# per-chunk stored stride, %16==0 so fp8 DoubleRow APs are 16B-aligned
SLAB_STRIDE = [((c + 15) // 16) * 16 for c in SLAB_NCOL]
SLAB_OFF = [0]
for j in range(NSLAB - 1):
    SLAB_OFF.append(SLAB_OFF[-1] + P * NCHUNK * SLAB_STRIDE[j])
XT_SIZE = SLAB_OFF[-1] + P * NCHUNK * SLAB_STRIDE[-1]

_cache = {}


def _install_ntff_hook():
    """Best-effort: make antenv.axon_hooks importable so that
    run_bass_kernel_spmd's trace path (e.g. via BASS_TRACE=1) degrades
    gracefully instead of raising ImportError.  Registers the real
    libaxon NTFF hook when available, else a no-op."""
    import sys
    import types

    try:
        import antenv.axon_hooks  # noqa: F401

        return
    except ImportError:
        pass
    try:
        import antenv
    except ImportError:
        return
    mod = types.ModuleType("antenv.axon_hooks")
    mod._hook = None
    mod.set_axon_ntff_profile_hook = lambda h: setattr(mod, "_hook", h)
    mod.get_axon_ntff_profile_hook = lambda: mod._hook
    sys.modules["antenv.axon_hooks"] = mod
    antenv.axon_hooks = mod
    try:
        if "/root/.axon_site" not in sys.path:
            sys.path.insert(0, "/root/.axon_site")
        from trn_agent_boot.trn_boot import _ntff_profile_via_ctypes

        mod._hook = _ntff_profile_via_ctypes("/opt/axon/libaxon_pjrt.so")
    except Exception:
        pass


def _build():
    import concourse.bass as bass
    import concourse.bacc as bacc
    import concourse.tile as tile
    from concourse import mybir

    nc = bacc.Bacc("TRN2", target_bir_lowering=False, debug=False)
    f32 = mybir.dt.float32
    in_dt = getattr(mybir.dt, IN_DT)

    xt_d = nc.dram_tensor("xt", [XT_SIZE], in_dt, kind="ExternalInput").ap()
    bf16 = mybir.dt.bfloat16
    mk_d = nc.dram_tensor("mk", [P, 2 * 2 * W], bf16, kind="ExternalInput").ap()
    sc_d = nc.dram_tensor("sc", [2, P, NBLK], f32, kind="ExternalOutput").ap()

    with tile.TileContext(nc) as tc:
        with (
            tc.tile_pool(name="xin", bufs=1) as xin,
            tc.tile_pool(name="cst", bufs=1) as cst,
            tc.tile_pool(name="scr", bufs=4) as scr,
            tc.tile_pool(name="outp", bufs=1) as outp,
            tc.tile_pool(name="psum", bufs=8, space=bass.MemorySpace.PSUM) as psum,
        ):
            mk = cst.tile([P, 2 * 2 * W], bf16, name="mk_sb")
            sc = outp.tile([P, 2 * NBLK], f32, name="sc_sb")

            # One DMA per slab, all on one HWDGE queue so transfers finish
            # in issue order (slab 0 first, then the mask, then the rest).
            xt = []
            for j in range(NSLAB):
                t = xin.tile(
                    [P, NCHUNK * SLAB_STRIDE[j]], in_dt,
                    tag=f"xt_{j}", name=f"xt_{j}",
                )
                n = P * NCHUNK * SLAB_STRIDE[j]
                src_ap = xt_d[SLAB_OFF[j] : SLAB_OFF[j] + n].rearrange(
                    "(p c) -> p c", p=P
                )
                nc.sync.dma_start(out=t, in_=src_ap)
                xt.append(t)
                if j == 0:
                    nc.sync.dma_start(out=mk, in_=mk_d)

            # Blocks are processed in PSUM pairs (one bank holds two
            # Gram blocks).
            groups = [(2 * t, 2 * t + 1) for t in range(NBLK // 2)]
            for gi, grp in enumerate(groups):
                gw = len(grp) * W
                pb = psum.tile([P, gw], f32, tag="gram", name=f"gram_{gi}")
                for u, i in enumerate(grp):
                    j = max(jj for jj in range(NSLAB) if SLAB_FIRST[jj] <= i)
                    m0 = (i - SLAB_FIRST[j]) * P
                    t = xt[j]
                    out_ap = pb[:, u * W : (u + 1) * W]
                    if DOUBLE_ROW:
                        t3 = t[:].rearrange("p (c w) -> p c w", w=SLAB_STRIDE[j])
                        for k in range(NCHUNK // 2):
                            lhsT = t3[:, 2 * k : 2 * k + 2, m0 : m0 + P]
                            rhs = t3[:, 2 * k : 2 * k + 2, m0 : m0 + W]
                            nc.tensor.matmul(
                                out_ap, lhsT, rhs,
                                start=(k == 0),
                                stop=(k == NCHUNK // 2 - 1),
                                perf_mode=mybir.MatmulPerfMode.DoubleRow,
                            )
                    else:
                        for k in range(NCHUNK):
                            c0 = k * SLAB_STRIDE[j] + m0
                            nc.tensor.matmul(
                                out_ap,
                                t[:, c0 : c0 + P],
                                t[:, c0 : c0 + W],
                                start=(k == 0),
                                stop=(k == NCHUNK - 1),
                            )
                nb = len(grp)
                for h in range(2):  # 0 -> diag (s), 1 -> superdiag (c)
                    tmp = scr.tile(
                        [P, 2 * W], f32, tag="scr", name=f"scr_{gi}_{h}"
                    )
                    col = 2 * grp[0] + 2 * h  # [pair, h, u] layout
                    nc.vector.tensor_mul(
                        tmp[:, :gw], pb, mk[:, 2 * h * W : 2 * h * W + gw]
                    )
                    nc.vector.reduce_sum(
                        sc[:, col : col + nb],
                        tmp[:, :gw].rearrange("p (b c) -> p b c", b=nb),
                        axis=mybir.AxisListType.X,
                    )

                if gi == NBLK // 4 - 1:
                    nc.sync.dma_start(out=sc_d[0], in_=sc[:, :NBLK])
            nc.sync.dma_start(out=sc_d[1], in_=sc[:, NBLK:])
    nc.compile()
    return nc


def _make_masks():
    mk = np.zeros((P, 2, 2, W), np.float32)
    r = np.arange(P)
    mk[r, 0, :, r] = 1.0      # diag mask, replicated for both blocks
    mk[r, 1, :, r + 1] = 1.0  # superdiag mask
    return mk.reshape(P, 2 * 2 * W).astype(ml_dtypes.bfloat16)


def _np_in_dt():
    return {"float8e4": ml_dtypes.float8_e4m3, "bfloat16": ml_dtypes.bfloat16}[IN_DT]


def _prep_inputs(x):
    """x: [B, L, D] float32 -> (list of per-core input maps, scales)."""
    np_dt = _np_in_dt()
    mk = _make_masks()
    in_maps = []
    scales = []
    for b in range(B):
        # Scale into fp8 range (the loss is scale-invariant; the exact
        # factor is divided back out of s and c on the host).
        amax = float(np.max(np.abs(x[b])))
        alpha = (200.0 / amax) if (np.isfinite(amax) and amax > 0) else 1.0
        scales.append(alpha)
        xs = x[b].T * np.float32(alpha)
        xt = np.zeros((D, L + 1), dtype=np_dt)
        xt[:, :L] = np.ascontiguousarray(xs).astype(np_dt)
        arr = np.zeros(XT_SIZE, dtype=np_dt)
        for j in range(NSLAB):
            st, ncol = SLAB_STRIDE[j], SLAB_NCOL[j]
            r0 = SLAB_FIRST[j] * P
            sl = xt[:, r0 : r0 + ncol]                       # [D, ncol]
            a4 = arr[SLAB_OFF[j] : SLAB_OFF[j] + P * NCHUNK * st].reshape(
                P, NCHUNK, st
            )
            a4[:, :, :ncol] = sl.reshape(NCHUNK, P, ncol).transpose(1, 0, 2)
        in_maps.append({"xt": arr, "mk": mk})
    return in_maps, scales


def _combine(results, scales):
    total = 0.0
    for b in range(B):
        sc = np.asarray(results[b]["sc"], dtype=np.float64)  # [2, P, NBLK]
        sc = sc / (scales[b] * scales[b])
        q = sc.transpose(1, 0, 2).reshape(P, 2 * NBLK).reshape(
            P, NBLK // 2, 2, 2
        )  # [p, pair, h, u]
        s = q[:, :, 0, :].transpose(1, 2, 0).reshape(-1)  # l = 128*i + p
        c = q[:, :, 1, :].transpose(1, 2, 0).reshape(-1)
        n = np.maximum(np.sqrt(s), EPS)
        diag = c[: L - 1] / (n[: L - 1] * n[1:L])
        total += diag.sum()
    coherence = total / (B * (L - 1))
    return np.array(1.0 - coherence, dtype=np.float32)


def _run(x, trace=False):
    from concourse import bass_utils

    _install_ntff_hook()
    if "nc" not in _cache:
        _cache["nc"] = _build()
    nc = _cache["nc"]
    in_maps, scales = _prep_inputs(np.asarray(x, dtype=np.float32))
    res = bass_utils.run_bass_kernel_spmd(
        nc, in_maps, core_ids=list(range(B)), trace=trace
    )
    return _combine(res.results, scales), res


def kernel(hidden_states):
    out, _ = _run(hidden_states, trace=False)
    return out
